# revision 1
# baseline (speedup 1.0000x reference)
"""GPT2 attention (B=2,S=2048,E=1024,H=16) on 8 NeuronCores.

Sharding: core c -> batch b=c//4, head-group g=c%4 (4 heads, d'=256 cols).
Per core (all matmuls in float32r: full PE rate with free dim >=256, fp32 data):
  - Q^T,K^T = (Wqk^T @ x)            [d,s] layout, bias per-partition (DVE)
  - V       = (x @ Wv_aug)           [s,d] layout, 65-col-per-head blocks with a
                                     ones column; bias added via broadcast tile
  - S^T     = K^T.T @ Q^T            [k,q] blocks (128k x 512q), causal block-skip
  - exp     on ACT over paired k-tile slabs [128, 1024]
  - mask    diagonal blocks: memset above-diag region + triangular mul (DVE)
  - O^T,Z   = V_aug.T @ expS^T       [65, 512] psum; row 64 = softmax denom Z
  - norm    recipZ (DVE) -> PE broadcast to 64 rows -> DVE mul -> attnT [256,2048]
  - partial c_proj = attnT.T @ Wp    [s, e], summed across head-groups on host
"""

import numpy as np

import concourse.bass as bass
import concourse.mybir as mybir
import concourse.tile as tile
from concourse import bacc
from concourse.bass_utils import run_bass_kernel_spmd

B, S, E, H = 2, 2048, 1024, 16
HD = 64           # head dim
HPC = 4           # heads per core
DP = HPC * HD     # 256 d' columns per core
NQC = 4           # q-chunks of 512
QCW = 512         # q-chunk width
NKT = S // 128    # 16 k-tiles
NST = S // 128    # 16 s-tiles
NET = E // 128    # 8 E-tiles

f32 = mybir.dt.float32
f32r = mybir.dt.float32r
bf16 = mybir.dt.bfloat16
FT = mybir.ActivationFunctionType

_CACHED = {}


def build_nc():
    nc = bacc.Bacc("TRN2", target_bir_lowering=False, debug=False,
                   enable_asserts=False, num_devices=8)

    xT = nc.dram_tensor("xT", [E, S], bf16, kind="ExternalInput")
    wqk = nc.dram_tensor("wqk", [E, 2 * DP], bf16, kind="ExternalInput")
    bqk = nc.dram_tensor("bqk", [128, 4], f32, kind="ExternalInput")
    wv = nc.dram_tensor("wv", [E, 260], bf16, kind="ExternalInput")
    vb = nc.dram_tensor("vb", [128, 260], f32, kind="ExternalInput")
    wp = nc.dram_tensor("wp", [DP, E], bf16, kind="ExternalInput")
    mask4 = nc.dram_tensor("mask4", [128, 2048], bf16, kind="ExternalInput")
    on = nc.dram_tensor("on", [1, 128], f32r, kind="ExternalInput")
    outp = nc.dram_tensor("outp", [S, E], f32, kind="ExternalOutput")

    with tile.TileContext(nc) as tc:
        with (
            nc.allow_low_precision("float32r is 4-byte fp32-layout data"),
            tc.tile_pool(name="consts", bufs=1) as consts,
            tc.tile_pool(name="acts", bufs=1) as acts,
            tc.tile_pool(name="slabs", bufs=5) as slabs,
            tc.tile_pool(name="small", bufs=3) as small,
            tc.tile_pool(name="outs", bufs=3) as outs,
            tc.tile_pool(name="ps", bufs=3, space="PSUM") as ps,
            tc.tile_pool(name="otps", bufs=2, space="PSUM") as otps,
        ):
            # ---- constants / weights in ----
            xt_sb = []
            wqk_sb = []
            wv_sb = []
            for t in range(NET):
                xt = consts.tile([128, S], bf16, tag=f"xt{t}")
                nc.sync.dma_start(xt[:], xT[t * 128:(t + 1) * 128, :])
                xt_sb.append(xt)
                wq = consts.tile([128, 2 * DP], bf16, tag=f"wqk{t}")
                nc.sync.dma_start(wq[:], wqk[t * 128:(t + 1) * 128, :])
                wqk_sb.append(wq)
                wvt = consts.tile([128, 260], bf16, tag=f"wv{t}")
                nc.sync.dma_start(wvt[:], wv[t * 128:(t + 1) * 128, :])
                wv_sb.append(wvt)
            vb_sb = consts.tile([128, 260], f32, tag="vb")
            nc.sync.dma_start(vb_sb[:], vb[:, :])
            bqk_sb = consts.tile([128, 4], f32, tag="bqk")
            nc.sync.dma_start(bqk_sb[:], bqk[:, :])
            wp_sb = []
            for t in range(2):
                wpt = consts.tile([128, E], bf16, tag=f"wp{t}")
                nc.sync.dma_start(wpt[:], wp[t * 128:(t + 1) * 128, :])
                wp_sb.append(wpt)
            mask4_sb = consts.tile([128, 2048], bf16, tag="mask4")
            nc.sync.dma_start(mask4_sb[:], mask4[:, :])
            on_sb = consts.tile([1, 128], f32r, tag="on")
            nc.sync.dma_start(on_sb[:], on[:, :])

            # ---- persistent activations ----
            v_sb = [acts.tile([128, 260], bf16, tag=f"v{st}", name=f"v{st}") for st in range(NST)]
            qkt_sb = [acts.tile([128, S], bf16, tag=f"qkt{t}", name=f"qkt{t}") for t in range(4)]
            attnT_sb = [acts.tile([128, S], bf16, tag=f"attnT{t}", name=f"attnT{t}") for t in range(2)]

            # ---- V projection: V_aug[s, 260] = x @ Wv_aug + vb ----
            for st in range(NST):
                vps = ps.tile([128, 260], f32, tag="ps")
                for kt in range(NET):
                    nc.tensor.matmul(
                        vps[:],
                        (xt_sb[kt][:, st * 128:(st + 1) * 128]),
                        (wv_sb[kt][:]),
                        start=(kt == 0), stop=(kt == NET - 1),
                    )
                nc.vector.tensor_add(v_sb[st][:], vps[:], vb_sb[:])

            # ---- QK^T projection: [d, s] = Wqk.T @ x (+bias per-partition) ----
            for t in (0, 2, 1, 3):
                for sc in range(4):
                    qps = ps.tile([128, 512], f32, tag="ps")
                    for kt in range(NET):
                        nc.tensor.matmul(
                            qps[:],
                            (wqk_sb[kt][:, t * 128:(t + 1) * 128]),
                            (xt_sb[kt][:, sc * 512:(sc + 1) * 512]),
                            start=(kt == 0), stop=(kt == NET - 1),
                        )
                    nc.vector.tensor_scalar_add(
                        qkt_sb[t][:, sc * 512:(sc + 1) * 512],
                        qps[:], bqk_sb[:, t:t + 1])

            # ---- attention: head pairs interleaved at k-pair level ----
            for qc in range(NQC):
                nkt = 4 * qc + 4  # causal: k-tiles 0 .. 4qc+3
                for hp in range(2):
                    ots = [otps.tile([65, 512], f32, tag="ot",
                                     name=f"ot{qc}_{hp}_{i}") for i in range(2)]
                    for kp in range(nkt // 2):  # k-tile pairs
                        for i in range(2):
                            h = 2 * hp + i
                            tq = h // 2
                            po = (h % 2) * 64
                            qt_ap = qkt_sb[tq]
                            kt_ap = qkt_sb[2 + tq]
                            sp = ps.tile([128, 1024], f32, tag="ps")
                            for half in range(2):
                                kt = 2 * kp + half
                                nc.tensor.matmul(
                                    sp[:, half * 512:(half + 1) * 512],
                                    (kt_ap[po:po + 64, kt * 128:(kt + 1) * 128]),
                                    (qt_ap[po:po + 64, qc * 512:(qc + 1) * 512]),
                                    start=True, stop=True,
                                )
                            slab = slabs.tile([128, 1024], bf16, tag="slab")
                            nc.scalar.activation(slab[:], sp[:], FT.Exp)
                            for half in range(2):
                                kt = 2 * kp + half
                                di = kt - 4 * qc  # diagonal sub-position
                                if di >= 0:
                                    base = half * 512
                                    nc.vector.tensor_mul(
                                        slab[:, base:base + 512],
                                        slab[:, base:base + 512],
                                        mask4_sb[:, di * 512:(di + 1) * 512])
                            for half in range(2):
                                kt = 2 * kp + half
                                nc.tensor.matmul(
                                    ots[i][:],
                                    (v_sb[kt][:, 65 * h:65 * h + 65]),
                                    (slab[:, half * 512:(half + 1) * 512]),
                                    start=(kt == 0), stop=(kt == nkt - 1),
                                )
                    # normalize: rows 0..63 * (1/Z), Z = row 64
                    for i in range(2):
                        h = 2 * hp + i
                        po = (h % 2) * 64
                        zrow = small.tile([1, 512], f32, tag="zrow")
                        nc.vector.tensor_copy(zrow[:], ots[i][64:65, :])
                        rz = small.tile([1, 512], f32, tag="rz")
                        nc.vector.reciprocal_approx_fast(rz[:], zrow[:])
                        sbb = small.tile([64, 512], f32, tag="sbb")
                        nc.gpsimd.partition_broadcast(sbb[:], rz[0:1, :])
                        nc.vector.tensor_mul(
                            attnT_sb[h // 2][po:po + 64,
                                             qc * 512:(qc + 1) * 512],
                            ots[i][0:64, :], sbb[:])

                # ---- c_proj for the 4 s-tiles of this q-chunk ----
                for sti in range(4):
                    st = 4 * qc + sti
                    for nchk in range(2):
                        cps = ps.tile([128, 512], f32, tag="ps")
                        for kt2 in range(2):
                            nc.tensor.matmul(
                                cps[:],
                                (attnT_sb[kt2][:, st * 128:(st + 1) * 128]),
                                (wp_sb[kt2][:, nchk * 512:(nchk + 1) * 512]),
                                start=(kt2 == 0), stop=(kt2 == 1),
                            )
                        ob = outs.tile([128, 512], f32, tag="ob")
                        nc.vector.tensor_copy(ob[:], cps[:])
                        nc.sync.dma_start(
                            outp[st * 128:(st + 1) * 128,
                                 nchk * 512:(nchk + 1) * 512], ob[:])

    nc.finalize()
    return nc


def _prep_inputs(hidden_states, w_attn, b_attn, w_proj, b_proj):
    hs = np.asarray(hidden_states, np.float32)
    wa = np.asarray(w_attn, np.float32)
    ba = np.asarray(b_attn, np.float32)
    wpj = np.asarray(w_proj, np.float32)

    import ml_dtypes
    bfl = ml_dtypes.bfloat16
    xTs = [np.ascontiguousarray(hs[b].T.astype(bfl)) for b in range(B)]
    triu = (np.arange(128)[:, None] <= np.arange(128)[None, :]).astype(np.float32)
    mask4 = np.zeros((128, 2048), np.float32)
    for i in range(4):
        m = np.ones((128, 512), np.float32)
        m[:, :i * 128] = 0.0
        m[:, i * 128:(i + 1) * 128] = triu
        mask4[:, i * 512:(i + 1) * 512] = m
    mask4 = mask4.astype(ml_dtypes.bfloat16)
    on = np.ones((1, 128), np.float32)

    in_maps = []
    for c in range(8):
        b, g = c // 4, c % 4
        q0 = DP * g
        k0 = E + DP * g
        v0 = 2 * E + DP * g
        wqk = np.concatenate(
            [wa[:, q0:q0 + DP] * 0.125, wa[:, k0:k0 + DP]], axis=1).astype(bfl)
        bqk = np.zeros((128, 4), np.float32)
        bqk[:, 0] = ba[q0:q0 + 128] * 0.125
        bqk[:, 1] = ba[q0 + 128:q0 + 256] * 0.125
        bqk[:, 2] = ba[k0:k0 + 128]
        bqk[:, 3] = ba[k0 + 128:k0 + 256]
        wv = np.zeros((E, 260), bfl)
        vb = np.zeros((128, 260), np.float32)
        for h in range(HPC):
            wv[:, 65 * h:65 * h + 64] = wa[:, v0 + 64 * h:v0 + 64 * h + 64].astype(bfl)
            vb[:, 65 * h:65 * h + 64] = ba[v0 + 64 * h:v0 + 64 * h + 64]
            vb[:, 65 * h + 64] = 1.0
        wp = np.ascontiguousarray(wpj[DP * g:DP * (g + 1), :].astype(bfl))
        in_maps.append({
            "xT": xTs[b],
            "wqk": np.ascontiguousarray(wqk),
            "bqk": bqk,
            "wv": wv,
            "vb": vb,
            "wp": wp,
            "mask4": mask4,
            "on": on,
        })
    return in_maps


def run(trace=False, **inputs):
    if "nc" not in _CACHED:
        _CACHED["nc"] = build_nc()
    nc = _CACHED["nc"]
    in_maps = _prep_inputs(**inputs)
    res = run_bass_kernel_spmd(nc, in_maps, list(range(8)), trace=trace)
    b_proj = np.asarray(inputs["b_proj"], np.float32)
    out = np.empty((B, S, E), np.float32)
    for b in range(B):
        acc = res.results[4 * b]["outp"].astype(np.float32)
        for g in range(1, 4):
            acc = acc + res.results[4 * b + g]["outp"]
        out[b] = acc + b_proj
    return out, res


def kernel(**inputs):
    out, _ = run(trace=False, **inputs)
    return out



# revision 3
# speedup vs baseline: 1.4680x; 1.4680x over previous
"""GPT2 attention (B=2,S=2048,E=1024,H=16) on 8 NeuronCores.

Sharding: core c -> batch b=c//4, head-group g=c%4 (4 heads, d'=256 cols).

Schedule (v2): ACT-paced software pipeline. Per q-chunk (512 q), the two
head pairs run sequential k-pair chains; scores MM pairs are emitted
adjacent so the auto-derived tile_position (0,0)/(64,0) row-tiles run
concurrently on the PE; attn@V lags one step behind exp; QKV-proj and
c_proj matmuls are drip-fed as PE filler between steps. Causal structure
is trimmed at 128-col granularity (scores/exp/mask/attn@V all skip the
fully-masked region of diagonal k-tiles). Partial c_proj outputs are
written bf16 and summed on host.
"""

import numpy as np

import concourse.bass as bass
import concourse.mybir as mybir
import concourse.tile as tile
from concourse import bacc
from concourse.bass_utils import run_bass_kernel_spmd

B, S, E, H = 2, 2048, 1024, 16
HD = 64           # head dim
HPC = 4           # heads per core
DP = HPC * HD     # 256 d' columns per core
NQC = 4           # q-chunks of 512
QCW = 512         # q-chunk width
NST = S // 128    # 16 s-tiles
NET = E // 128    # 8 E-tiles

f32 = mybir.dt.float32
bf16 = mybir.dt.bfloat16
FT = mybir.ActivationFunctionType

_CACHED = {}


def build_nc():
    nc = bacc.Bacc("TRN2", target_bir_lowering=False, debug=False,
                   enable_asserts=False, num_devices=8)

    xT = nc.dram_tensor("xT", [E, S], bf16, kind="ExternalInput")
    wqk = nc.dram_tensor("wqk", [E, 2 * DP], bf16, kind="ExternalInput")
    bqk = nc.dram_tensor("bqk", [128, 4], f32, kind="ExternalInput")
    wv = nc.dram_tensor("wv", [E, 260], bf16, kind="ExternalInput")
    vb = nc.dram_tensor("vb", [128, 260], f32, kind="ExternalInput")
    wp = nc.dram_tensor("wp", [DP, E], bf16, kind="ExternalInput")
    mtri = nc.dram_tensor("mtri", [128, 128], bf16, kind="ExternalInput")
    outp = nc.dram_tensor("outp", [S, E], bf16, kind="ExternalOutput")

    with tile.TileContext(nc) as tc:
        with (
            nc.allow_low_precision("bf16 data with fp32 psum accumulation"),
            tc.tile_pool(name="consts", bufs=1) as consts,
            tc.tile_pool(name="acts", bufs=1) as acts,
            tc.tile_pool(name="slabs", bufs=5) as slabs,
            tc.tile_pool(name="small", bufs=4) as small,
            tc.tile_pool(name="outs", bufs=3) as outs,
            tc.tile_pool(name="scps", bufs=2, space="PSUM") as scps,
            tc.tile_pool(name="otps", bufs=2, space="PSUM") as otps,
            tc.tile_pool(name="fps", bufs=2, space="PSUM") as fps,
        ):
            # ---- constants / weights in (order matters: wqk+xT first) ----
            wqk_sb = []
            for t in range(NET):
                wq = consts.tile([128, 2 * DP], bf16, tag=f"wqk{t}")
                nc.sync.dma_start(wq[:], wqk[t * 128:(t + 1) * 128, :])
                wqk_sb.append(wq)
            xt_sb = []
            for t in range(NET):
                xt = consts.tile([128, S], bf16, tag=f"xt{t}")
                nc.sync.dma_start(xt[:], xT[t * 128:(t + 1) * 128, :])
                xt_sb.append(xt)
            bqk_sb = consts.tile([128, 4], f32, tag="bqk")
            nc.sync.dma_start(bqk_sb[:], bqk[:, :])
            wv_sb = []
            for t in range(NET):
                wvt = consts.tile([128, 260], bf16, tag=f"wv{t}")
                nc.sync.dma_start(wvt[:], wv[t * 128:(t + 1) * 128, :])
                wv_sb.append(wvt)
            vb_sb = consts.tile([128, 260], f32, tag="vb")
            nc.sync.dma_start(vb_sb[:], vb[:, :])
            mtri_sb = consts.tile([128, 128], bf16, tag="mtri")
            nc.sync.dma_start(mtri_sb[:], mtri[:, :])
            wp_sb = []
            for t in range(2):
                wpt = consts.tile([128, E], bf16, tag=f"wp{t}")
                nc.sync.dma_start(wpt[:], wp[t * 128:(t + 1) * 128, :])
                wp_sb.append(wpt)

            # ---- persistent activations ----
            v_sb = [acts.tile([128, 260], bf16, tag=f"v{st}", name=f"v{st}")
                    for st in range(NST)]
            qkt_sb = [acts.tile([128, S], bf16, tag=f"qkt{t}", name=f"qkt{t}")
                      for t in range(4)]
            attnT_sb = [acts.tile([128, S], bf16, tag=f"attnT{t}",
                                  name=f"attnT{t}") for t in range(2)]

            # ================= filler units (PE work drip-fed) ==========
            def emit_kq(t, sc):
                """QK^T proj: qkt_sb[t][:, sc*512:(sc+1)*512] (+bias)."""
                qps = fps.tile([128, 512], f32, tag="fp", name="qps")
                for kt in range(NET):
                    nc.tensor.matmul(
                        qps[:],
                        wqk_sb[kt][:, t * 128:(t + 1) * 128],
                        xt_sb[kt][:, sc * 512:(sc + 1) * 512],
                        start=(kt == 0), stop=(kt == NET - 1),
                    )
                nc.vector.tensor_scalar_add(
                    qkt_sb[t][:, sc * 512:(sc + 1) * 512],
                    qps[:], bqk_sb[:, t:t + 1])

            def emit_v(st):
                """V_aug[s-tile st] = x @ Wv_aug + vb (ones col via vb)."""
                vps = fps.tile([128, 512], f32, tag="fp", name="vps")
                for kt in range(NET):
                    nc.tensor.matmul(
                        vps[:, 0:260],
                        xt_sb[kt][:, st * 128:(st + 1) * 128],
                        wv_sb[kt][:],
                        start=(kt == 0), stop=(kt == NET - 1),
                    )
                nc.vector.tensor_add(v_sb[st][:], vps[:, 0:260], vb_sb[:])

            def emit_cp(st):
                """c_proj partial for s-tile st -> DRAM (bf16)."""
                ob = outs.tile([128, E], bf16, tag="ob", name="ob")
                for nchk in range(2):
                    cps = fps.tile([128, 512], f32, tag="fp", name="cps")
                    for kt2 in range(2):
                        nc.tensor.matmul(
                            cps[:],
                            attnT_sb[kt2][:, st * 128:(st + 1) * 128],
                            wp_sb[kt2][:, nchk * 512:(nchk + 1) * 512],
                            start=(kt2 == 0), stop=(kt2 == 1),
                        )
                    nc.vector.tensor_copy(
                        ob[:, nchk * 512:(nchk + 1) * 512], cps[:])
                nc.sync.dma_start(outp[st * 128:(st + 1) * 128, :], ob[:])

            queue = []
            emitted = set()

            def push(kind, a, b=None):
                queue.append((kind, a, b))

            def do_emit(u):
                if u in emitted:
                    return
                emitted.add(u)
                kind, a, b = u
                if kind == "KQ":
                    emit_kq(a, b)
                elif kind == "V":
                    emit_v(a)
                else:
                    emit_cp(a)

            def ensure(u):
                if u not in emitted:
                    do_emit(u)

            def pop_one():
                while queue:
                    u = queue.pop(0)
                    if u not in emitted:
                        do_emit(u)
                        return

            # ================= attention units ==========================
            def emit_scores(qc, pair, kp, sps, slbs):
                """S^T[k, q] for both heads of the pair; row-tiled MM pairs
                emitted adjacent. Returns psum tiles per head."""
                qt_ap = qkt_sb[pair]
                kt_ap = qkt_sb[2 + pair]
                for half in range(2):
                    kt = 2 * kp + half
                    di = kt - 4 * qc
                    q0 = max(di, 0) * 128  # first valid q col in chunk
                    for i in range(2):  # adjacent -> concurrent row tiles
                        po = i * 64
                        nc.tensor.matmul(
                            sps[i][:, half * 512 + q0:(half + 1) * 512],
                            kt_ap[po:po + 64, kt * 128:(kt + 1) * 128],
                            qt_ap[po:po + 64, qc * 512 + q0:(qc + 1) * 512],
                            start=True, stop=True,
                        )

            def emit_exp(qc, pair, kp, sps, slbs):
                for i in range(2):
                    diag = (2 * kp + 1 - 4 * qc) >= 0
                    if not diag:
                        nc.scalar.activation(slbs[i][:], sps[i][:], FT.Exp)
                    else:
                        for half in range(2):
                            kt = 2 * kp + half
                            q0 = max(kt - 4 * qc, 0) * 128
                            c0 = half * 512 + q0
                            c1 = (half + 1) * 512
                            nc.scalar.activation(
                                slbs[i][:, c0:c1], sps[i][:, c0:c1], FT.Exp)

            def emit_mask(qc, pair, kp, slbs):
                for half in range(2):
                    kt = 2 * kp + half
                    di = kt - 4 * qc
                    if di >= 0:
                        c0 = half * 512 + di * 128
                        for i in range(2):
                            nc.vector.tensor_mul(
                                slbs[i][:, c0:c0 + 128],
                                slbs[i][:, c0:c0 + 128], mtri_sb[:])

            def emit_av(qc, pair, kp, slbs, ots, nkt):
                for half in range(2):
                    kt = 2 * kp + half
                    di = kt - 4 * qc
                    q0 = max(di, 0) * 128
                    ensure(("V", kt, None))
                    for i in range(2):
                        hl = 2 * pair + i
                        nc.tensor.matmul(
                            ots[i][:, q0:512],
                            v_sb[kt][:, 65 * hl:65 * hl + 65],
                            slbs[i][:, half * 512 + q0:(half + 1) * 512],
                            start=(kt == 0), stop=(kt == nkt - 1),
                        )

            def emit_norm(qc, pair, ots):
                for i in range(2):
                    hl = 2 * pair + i
                    po = (hl % 2) * 64
                    zrow = small.tile([1, 512], f32, tag="zrow", name="zrow")
                    nc.vector.tensor_copy(zrow[:], ots[i][64:65, :])
                    rz = small.tile([1, 512], f32, tag="rz", name="rz")
                    nc.vector.reciprocal_approx_fast(rz[:], zrow[:])
                    sbb = small.tile([64, 512], f32, tag="sbb", name="sbb")
                    nc.gpsimd.partition_broadcast(sbb[:], rz[0:1, :])
                    nc.vector.tensor_mul(
                        attnT_sb[hl // 2][po:po + 64,
                                          qc * 512:(qc + 1) * 512],
                        ots[i][0:64, :], sbb[:])

            # ================= the schedule =============================
            # filler queue in rough need-order
            for u in [("KQ", 3, 0), ("KQ", 1, 0), ("V", 0, None),
                      ("V", 1, None), ("V", 2, None), ("V", 3, None),
                      ("KQ", 2, 1), ("KQ", 0, 1), ("V", 4, None),
                      ("V", 5, None), ("V", 6, None), ("V", 7, None),
                      ("KQ", 3, 1), ("KQ", 1, 1),
                      ("KQ", 2, 2), ("KQ", 0, 2),
                      ("V", 8, None), ("V", 9, None), ("V", 10, None),
                      ("V", 11, None),
                      ("KQ", 3, 2), ("KQ", 1, 2),
                      ("KQ", 2, 3), ("KQ", 0, 3),
                      ("V", 12, None), ("V", 13, None), ("V", 14, None),
                      ("V", 15, None),
                      ("KQ", 3, 3), ("KQ", 1, 3)]:
                push(*u)

            # head: minimal K/Q proj for qc0 pair0
            do_emit(("KQ", 2, 0))
            do_emit(("KQ", 0, 0))

            steps = []
            for qc in range(NQC):
                nkt = 4 * qc + 4
                for pair in range(2):
                    for kp in range(nkt // 2):
                        steps.append((qc, pair, kp, nkt))

            prev = None          # (qc, pair, kp, nkt, slbs, ots)
            ots_cur = None
            for (qc, pair, kp, nkt) in steps:
                if kp == 0:
                    ensure(("KQ", 2 + pair, qc))
                    ensure(("KQ", pair, qc))
                    ots_cur = [otps.tile([65, 512], f32, tag="ot",
                                         name=f"ot{qc}_{pair}_{i}")
                               for i in range(2)]
                sps = [scps.tile([128, 1024], f32, tag="sp",
                                 name=f"sp{qc}_{pair}_{kp}_{i}")
                       for i in range(2)]
                slbs = [slabs.tile([128, 1024], bf16, tag="slab",
                                   name=f"sl{qc}_{pair}_{kp}_{i}")
                        for i in range(2)]
                if prev is not None:
                    pqc, ppair, pkp, pnkt, pslbs, pots = prev
                    emit_av(pqc, ppair, pkp, pslbs, pots, pnkt)
                    if pkp == pnkt // 2 - 1:  # pair chain finished
                        emit_norm(pqc, ppair, pots)
                        if ppair == 1:  # whole qc finished -> c_proj
                            for st in range(4 * pqc, 4 * pqc + 4):
                                push("CP", st, None)
                emit_scores(qc, pair, kp, sps, slbs)
                emit_exp(qc, pair, kp, sps, slbs)
                emit_mask(qc, pair, kp, slbs)
                pop_one()
                prev = (qc, pair, kp, nkt, slbs, ots_cur)

            # tail
            pqc, ppair, pkp, pnkt, pslbs, pots = prev
            emit_av(pqc, ppair, pkp, pslbs, pots, pnkt)
            emit_norm(pqc, ppair, pots)
            for st in range(12, 16):
                push("CP", st, None)
            while queue:
                pop_one()

    nc.finalize()
    return nc


def _prep_inputs(hidden_states, w_attn, b_attn, w_proj, b_proj):
    hs = np.asarray(hidden_states, np.float32)
    wa = np.asarray(w_attn, np.float32)
    ba = np.asarray(b_attn, np.float32)
    wpj = np.asarray(w_proj, np.float32)

    import ml_dtypes
    bfl = ml_dtypes.bfloat16
    xTs = [np.ascontiguousarray(hs[b].T.astype(bfl)) for b in range(B)]
    mtri = (np.arange(128)[:, None] <= np.arange(128)[None, :]).astype(bfl)

    in_maps = []
    for c in range(8):
        b, g = c // 4, c % 4
        q0 = DP * g
        k0 = E + DP * g
        v0 = 2 * E + DP * g
        wqk = np.concatenate(
            [wa[:, q0:q0 + DP] * 0.125, wa[:, k0:k0 + DP]], axis=1).astype(bfl)
        bqk = np.zeros((128, 4), np.float32)
        bqk[:, 0] = ba[q0:q0 + 128] * 0.125
        bqk[:, 1] = ba[q0 + 128:q0 + 256] * 0.125
        bqk[:, 2] = ba[k0:k0 + 128]
        bqk[:, 3] = ba[k0 + 128:k0 + 256]
        wv = np.zeros((E, 260), bfl)
        vb = np.zeros((128, 260), np.float32)
        for h in range(HPC):
            wv[:, 65 * h:65 * h + 64] = \
                wa[:, v0 + 64 * h:v0 + 64 * h + 64].astype(bfl)
            vb[:, 65 * h:65 * h + 64] = ba[v0 + 64 * h:v0 + 64 * h + 64]
            vb[:, 65 * h + 64] = 1.0
        wp = np.ascontiguousarray(wpj[DP * g:DP * (g + 1), :].astype(bfl))
        in_maps.append({
            "xT": xTs[b],
            "wqk": np.ascontiguousarray(wqk),
            "bqk": bqk,
            "wv": wv,
            "vb": vb,
            "wp": wp,
            "mtri": mtri,
        })
    return in_maps


def run(trace=False, **inputs):
    if "nc" not in _CACHED:
        _CACHED["nc"] = build_nc()
    nc = _CACHED["nc"]
    in_maps = _prep_inputs(**inputs)
    res = run_bass_kernel_spmd(nc, in_maps, list(range(8)), trace=trace)
    b_proj = np.asarray(inputs["b_proj"], np.float32)
    out = np.empty((B, S, E), np.float32)
    for b in range(B):
        acc = res.results[4 * b]["outp"].astype(np.float32)
        for g in range(1, 4):
            acc = acc + res.results[4 * b + g]["outp"].astype(np.float32)
        out[b] = acc + b_proj
    return out, res


def kernel(**inputs):
    out, _ = run(trace=False, **inputs)
    return out
